# revision 5
# baseline (speedup 1.0000x reference)
"""Trainium2 Bass kernel for nn_DirectedHyperConvNetwork (3-layer hyper-graph
message passing: 6 COO SpMMs + residual + dropout + mean).

Strategy (8 NeuronCores, SPMD, one NEFF):
 - Each SpMM's destination rows are dealt degree-balanced into 8*T tiles of
   128 rows (host-side free row permutation; all index spaces remapped).
 - Per tile: bulk `dma_gather` pulls the source rows for ~C*128 edges into
   SBUF (edge i -> partition i%128, chunk i//128). Tables > 32767 rows are
   split at HALF for the int16 index limit (two gathers per tile).
 - Segment-sum on the PE: per 128-edge chunk, DVE builds
   W[p, r] = val[p] * (row[p] == r) with one fused tensor_scalar, and the PE
   accumulates W.T @ G into a PSUM tile over all chunks.
 - msg/x tables are republished between phases with an 8-core AllGather.
 - Dropout masks are bit-exact jax-CPU threefry, computed host-side (they
   depend only on the fixed key/shape) and passed as inputs.
 - Output mean accumulates into the (zero-initialized) output via SWDGE
   accumulate-DMA of 0.25-scaled tiles.
"""

import os
import subprocess
import sys
import tempfile

import numpy as np

P = 128
D = 256
CORES = 8
NUM_LAYERS = 3
KEEP = 0.9

# real problem dims
N_POIS = 50000
N_HYPER = 50000
NNZ = 1600000
HALF_DEFAULT = 32768


# --------------------------------------------------------------------------
# host-side preprocessing
# --------------------------------------------------------------------------

def _snake_pack(dest, n_rows, n_bins):
    """Deal rows of one destination space into n_bins bins of 128 rows,
    balancing edge counts. Returns perm[row] -> global padded position
    (bin*128 + round)."""
    deg = np.bincount(dest, minlength=n_rows)
    order = np.argsort(-deg, kind="stable")
    padded = np.full(n_bins * P, -1, np.int64)
    padded[:n_rows] = order
    rounds = padded.reshape(P, n_bins).copy()
    rounds[1::2] = rounds[1::2, ::-1]          # snake
    perm = np.empty(n_rows, np.int64)
    rr, bb = np.meshgrid(np.arange(P), np.arange(n_bins), indexing="ij")
    valid = rounds >= 0
    perm[rounds[valid]] = (bb * P + rr)[valid]
    return perm


def _layout_edges(dest_g, col_g, val, n_bins, half):
    """Assign edges to (bin, chunk, partition) slots.

    dest_g: per-edge global padded destination position.
    col_g: per-edge global padded gather index (into the permuted table).
    Returns (idxlo_w, idxhi_w, row_s, val_s, C_LO, C_HI):
      idxlo_w [128, n_bins, 8*C_LO] int16   (16-wrapped, 8x replicated)
      idxhi_w [128, n_bins, 8*C_HI] int16
      row_s   [128, n_bins, C] f32          (C = C_LO + C_HI)
      val_s   [128, n_bins, C] f32
    """
    bin_e = dest_g // P
    part_r = (dest_g % P).astype(np.float64)
    is_hi = col_g >= half
    key = bin_e * 2 + is_hi
    order = np.argsort(key, kind="stable")
    key_s = key[order]
    counts = np.bincount(key_s, minlength=n_bins * 2)
    starts = np.concatenate([[0], np.cumsum(counts)[:-1]])
    rank = np.arange(len(order)) - starts[key_s]       # rank within segment

    lo_cnt = counts[0::2]
    hi_cnt = counts[1::2]
    c_lo = max(1, int(np.ceil(lo_cnt.max() / P)))
    c_hi = max(1, int(np.ceil(hi_cnt.max() / P)))
    c_tot = c_lo + c_hi

    idxlo_flat = np.zeros((n_bins, c_lo * P), np.int16)
    idxhi_flat = np.zeros((n_bins, c_hi * P), np.int16)
    row_cv = np.zeros((n_bins, c_tot, P), np.float32)
    val_cv = np.zeros((n_bins, c_tot, P), np.float32)

    e = order
    ki = key_s
    lo_m = (ki % 2) == 0
    b_lo = ki[lo_m] // 2
    r_lo = rank[lo_m]
    idxlo_flat[b_lo, r_lo] = col_g[e[lo_m]].astype(np.int16)
    row_cv[b_lo, r_lo // P, r_lo % P] = part_r[e[lo_m]]
    val_cv[b_lo, r_lo // P, r_lo % P] = val[e[lo_m]]

    hi_m = ~lo_m
    b_hi = ki[hi_m] // 2
    r_hi = rank[hi_m]
    idxhi_flat[b_hi, r_hi] = (col_g[e[hi_m]] - half).astype(np.int16)
    row_cv[b_hi, c_lo + r_hi // P, r_hi % P] = part_r[e[hi_m]]
    val_cv[b_hi, c_lo + r_hi // P, r_hi % P] = val[e[hi_m]]

    def wrap(flat, c):
        # [n_bins, c*128] -> [128, n_bins, 8c]: element i of bin b at
        # [i%16, b, i//16], replicated over the 8 16-partition groups
        a = flat.reshape(n_bins, c * 8, 16).transpose(2, 0, 1)
        return np.tile(a, (8, 1, 1))

    idxlo_w = wrap(idxlo_flat, c_lo)
    idxhi_w = wrap(idxhi_flat, c_hi)
    row_s = row_cv.transpose(2, 0, 1).astype(np.float32)
    val_s = val_cv.transpose(2, 0, 1).astype(np.float32)
    return idxlo_w, idxhi_w, row_s, val_s, c_lo, c_hi


_MASK_CODE = """
import numpy as np, jax
import sys
n, d, layers = int(sys.argv[1]), int(sys.argv[2]), int(sys.argv[3])
key = jax.random.key(42)
out = np.empty((layers, n, d), np.bool_)
for i in range(layers):
    out[i] = np.asarray(jax.random.bernoulli(jax.random.fold_in(key, i), 0.9, (n, d)))
np.save(sys.argv[4], np.packbits(out, axis=-1))
"""


def _compute_masks(n, d, layers):
    """Dropout masks, bit-exact with the reference (jax threefry on CPU)."""
    with tempfile.TemporaryDirectory() as td:
        path = os.path.join(td, "m.npy")
        env = dict(os.environ)
        env["JAX_PLATFORMS"] = "cpu"
        subprocess.run(
            [sys.executable, "-c", _MASK_CODE, str(n), str(d), str(layers), path],
            check=True, env=env, capture_output=True,
        )
        packed = np.load(path)
    return np.unpackbits(packed, axis=-1).astype(bool)[:, :, :d]


# --------------------------------------------------------------------------
# device kernel builder
# --------------------------------------------------------------------------

def _build_nc(T, ct_lo, ct_hi, cs_lo, cs_hi, half, gtot, use_bf16):
    import concourse.bacc as bacc
    import concourse.mybir as mybir
    import concourse.tile as tile

    f32 = mybir.dt.float32
    gdt = mybir.dt.bfloat16 if use_bf16 else f32
    i16 = mybir.dt.int16
    RPC = T * P
    ct = ct_lo + ct_hi
    cs = cs_lo + cs_hi

    nc = bacc.Bacc("TRN2", target_bir_lowering=False, debug=False,
                   num_devices=CORES)

    def din(name, shape, dt):
        return nc.dram_tensor(name, shape, dt, kind="ExternalInput")

    x0_full = din("x0_full", [gtot, D], gdt)
    x0_slice = din("x0_slice", [RPC, D], f32)
    masks_d = din("masks", [NUM_LAYERS, RPC, D], f32)
    iota_d = din("iota", [P, P], gdt)
    it_lo = din("it_lo", [P, T * 8 * ct_lo], i16)
    it_hi = din("it_hi", [P, T * 8 * ct_hi], i16)
    rt_d = din("rt", [P, T * ct], gdt)
    vt_d = din("vt", [P, T * ct], gdt)
    is_lo = din("is_lo", [P, T * 8 * cs_lo], i16)
    is_hi = din("is_hi", [P, T * 8 * cs_hi], i16)
    rs_d = din("rs", [P, T * cs], gdt)
    vs_d = din("vs", [P, T * cs], gdt)

    out_acc = nc.dram_tensor("out_acc", [RPC, D], f32, kind="ExternalOutput")

    msg_loc = nc.dram_tensor("msg_loc", [RPC, D], gdt)
    msg_full = nc.dram_tensor("msg_full", [gtot, D], gdt, addr_space="Shared")
    xl_loc = nc.dram_tensor("xl_loc", [RPC, D], f32)
    x_full = nc.dram_tensor("x_full", [gtot, D], gdt, addr_space="Shared")
    if use_bf16:
        xl_locb = nc.dram_tensor("xl_locb", [RPC, D], gdt)
    else:
        xl_locb = xl_loc

    rg = [list(range(CORES))]

    with tile.TileContext(nc) as tc, \
         tc.tile_pool(name="res", bufs=1) as res, \
         tc.tile_pool(name="gp", bufs=2) as gp, \
         tc.tile_pool(name="wp", bufs=4) as wp, \
         tc.tile_pool(name="sm", bufs=3) as sm, \
         tc.tile_pool(name="pp", bufs=4, space="PSUM") as pp:

        # resident SBUF data
        iota_sb = res.tile([P, P], gdt)
        nc.sync.dma_start(iota_sb[:], iota_d[:, :])
        ed = {}
        for tag, (ilo, ihi, rr, vv, clo, chi, c) in {
            "t": (it_lo, it_hi, rt_d, vt_d, ct_lo, ct_hi, ct),
            "s": (is_lo, is_hi, rs_d, vs_d, cs_lo, cs_hi, cs),
        }.items():
            ilo_sb = res.tile([P, T * 8 * clo], i16, tag=f"ilo{tag}")
            ihi_sb = res.tile([P, T * 8 * chi], i16, tag=f"ihi{tag}")
            r_sb = res.tile([P, T * c], gdt, tag=f"r{tag}")
            v_sb = res.tile([P, T * c], gdt, tag=f"v{tag}")
            nc.sync.dma_start(ilo_sb[:], ilo[:, :])
            nc.sync.dma_start(ihi_sb[:], ihi[:, :])
            nc.sync.dma_start(r_sb[:], rr[:, :])
            nc.sync.dma_start(v_sb[:], vv[:, :])
            ed[tag] = (ilo_sb, ihi_sb, r_sb, v_sb, clo, chi, c)

        MAXC = 8     # max chunks (1024 indices) per dma_gather instruction

        def spmm_tile(tag, table, t, post):
            ilo_sb, ihi_sb, r_sb, v_sb, clo, chi, c = ed[tag]
            g = gp.tile([P, c, D], gdt, tag="g")
            for base, cnt, tab, idx_sb in (
                (0, clo, table[:half, :], ilo_sb),
                (clo, chi, table[half:, :], ihi_sb),
            ):
                cn = cnt
                for c0 in range(0, cn, MAXC):
                    cw = min(MAXC, cn - c0)
                    nc.gpsimd.dma_gather(
                        out_ap=g[:, base + c0:base + c0 + cw, :],
                        in_ap=tab,
                        idxs_ap=idx_sb[:, t * 8 * cn + 8 * c0:
                                       t * 8 * cn + 8 * (c0 + cw)],
                        num_idxs=cw * P,
                        num_idxs_reg=cw * P,
                        elem_size=D,
                    )
            ps = pp.tile([P, D], f32, space="PSUM", tag="ps")
            for k in range(c):
                w = wp.tile([P, P], gdt, tag="w")
                nc.vector.tensor_scalar(
                    out=w[:],
                    in0=iota_sb[:],
                    scalar1=r_sb[:, t * c + k:t * c + k + 1],
                    scalar2=v_sb[:, t * c + k:t * c + k + 1],
                    op0=mybir.AluOpType.is_equal,
                    op1=mybir.AluOpType.mult,
                )
                nc.tensor.matmul(
                    out=ps[:],
                    lhsT=w[:],
                    rhs=g[:, k, :],
                    start=(k == 0),
                    stop=(k == c - 1),
                )
            post(t, ps)

        for l in range(NUM_LAYERS):
            xtab = x0_full if l == 0 else x_full

            def post_tar(t, ps):
                msb = sm.tile([P, D], gdt, tag="msb")
                nc.vector.tensor_copy(msb[:], ps[:])
                nc.sync.dma_start(msg_loc[t * P:(t + 1) * P, :], msb[:])

            for t in range(T):
                spmm_tile("t", xtab, t, post_tar)

            nc.gpsimd.collective_compute(
                "AllGather", mybir.AluOpType.bypass, replica_groups=rg,
                ins=[msg_loc.ap().opt()], outs=[msg_full.ap().opt()],
            )

            def post_src(t, ps, l=l):
                xprev = sm.tile([P, D], f32, tag="xprev")
                src_prev = x0_slice if l == 0 else xl_loc
                nc.sync.dma_start(xprev[:], src_prev[t * P:(t + 1) * P, :])
                mk = sm.tile([P, D], f32, tag="mk")
                nc.sync.dma_start(mk[:], masks_d[l, t * P:(t + 1) * P, :])
                xn = sm.tile([P, D], f32, tag="xn")
                nc.vector.tensor_tensor(out=xn[:], in0=ps[:], in1=xprev[:],
                                        op=mybir.AluOpType.add)
                nc.vector.tensor_tensor(out=xn[:], in0=xn[:], in1=mk[:],
                                        op=mybir.AluOpType.mult)
                xq = sm.tile([P, D], f32, tag="xq")
                nc.vector.tensor_scalar_mul(xq[:], xn[:], 0.25)
                nc.gpsimd.dma_start(out=out_acc[t * P:(t + 1) * P, :],
                                    in_=xq[:], accum_op=mybir.AluOpType.add)
                if l == 0:
                    xq0 = sm.tile([P, D], f32, tag="xq0")
                    nc.vector.tensor_scalar_mul(xq0[:], xprev[:], 0.25)
                    nc.gpsimd.dma_start(out=out_acc[t * P:(t + 1) * P, :],
                                        in_=xq0[:],
                                        accum_op=mybir.AluOpType.add)
                if l < NUM_LAYERS - 1:
                    nc.sync.dma_start(xl_loc[t * P:(t + 1) * P, :], xn[:])
                    if use_bf16:
                        xnb = sm.tile([P, D], gdt, tag="xnb")
                        nc.vector.tensor_copy(xnb[:], xn[:])
                        nc.sync.dma_start(xl_locb[t * P:(t + 1) * P, :],
                                          xnb[:])

            for t in range(T):
                spmm_tile("s", msg_full, t, post_src)

            if l < NUM_LAYERS - 1:
                nc.gpsimd.collective_compute(
                    "AllGather", mybir.AluOpType.bypass, replica_groups=rg,
                    ins=[xl_locb.ap().opt()], outs=[x_full.ap().opt()],
                )

    nc.compile()
    return nc


# --------------------------------------------------------------------------
# public entry point
# --------------------------------------------------------------------------

def _run(poi_embs, src_row, src_col, src_val, tar_row, tar_col, tar_val,
         n_pois, n_hyper, use_bf16=True, trace=False):
    from concourse.bass_utils import run_bass_kernel_spmd

    n_bins_n = -(-n_pois // P)
    n_bins_n = -(-n_bins_n // CORES) * CORES     # multiple of CORES
    n_bins_h = -(-n_hyper // P)
    n_bins_h = -(-n_bins_h // CORES) * CORES
    n_bins = max(n_bins_n, n_bins_h)             # same T for both phases
    T = n_bins // CORES
    RPC = T * P
    gtot = n_bins * P
    half = min(HALF_DEFAULT, (gtot // 2 + 255) & ~255)

    perm_n = _snake_pack(src_row, n_pois, n_bins)    # POI space
    perm_h = _snake_pack(tar_row, n_hyper, n_bins)   # hyperedge space

    # tar-SpMM: dest in H-space, gathers from POI table
    it_lo, it_hi, rt, vt, ct_lo, ct_hi = _layout_edges(
        perm_h[tar_row], perm_n[tar_col], tar_val, n_bins, half)
    # src-SpMM: dest in N-space, gathers from hyperedge (msg) table
    is_lo, is_hi, rs, vs, cs_lo, cs_hi = _layout_edges(
        perm_n[src_row], perm_h[src_col], src_val, n_bins, half)

    x0_full = np.zeros((gtot, D), np.float32)
    x0_full[perm_n] = poi_embs
    masks = _compute_masks(n_pois, D, NUM_LAYERS)
    mask_scaled = np.zeros((NUM_LAYERS, gtot, D), np.float32)
    mask_scaled[:, perm_n] = masks.astype(np.float32) * np.float32(1.0 / KEEP)

    iota = np.broadcast_to(np.arange(P, dtype=np.float32), (P, P)).copy()

    if use_bf16:
        import ml_dtypes
        bdt = ml_dtypes.bfloat16
        x0_tab = x0_full.astype(bdt)
        iota = iota.astype(bdt)
        rt, vt, rs, vs = (a.astype(bdt) for a in (rt, vt, rs, vs))
    else:
        x0_tab = x0_full

    in_maps = []
    for c in range(CORES):
        bs = slice(c * T, (c + 1) * T)
        rows = slice(c * RPC, (c + 1) * RPC)
        in_maps.append({
            "x0_full": x0_tab,
            "x0_slice": x0_full[rows],
            "masks": np.ascontiguousarray(mask_scaled[:, rows]),
            "iota": iota,
            "it_lo": np.ascontiguousarray(it_lo[:, bs]).reshape(P, -1),
            "it_hi": np.ascontiguousarray(it_hi[:, bs]).reshape(P, -1),
            "rt": np.ascontiguousarray(rt[:, bs]).reshape(P, -1),
            "vt": np.ascontiguousarray(vt[:, bs]).reshape(P, -1),
            "is_lo": np.ascontiguousarray(is_lo[:, bs]).reshape(P, -1),
            "is_hi": np.ascontiguousarray(is_hi[:, bs]).reshape(P, -1),
            "rs": np.ascontiguousarray(rs[:, bs]).reshape(P, -1),
            "vs": np.ascontiguousarray(vs[:, bs]).reshape(P, -1),
        })

    nc = _build_nc(T, ct_lo, ct_hi, cs_lo, cs_hi, half, gtot, use_bf16)
    kw = {"trace": True} if trace else {}
    res = run_bass_kernel_spmd(nc, in_maps, core_ids=list(range(CORES)), **kw)

    full = np.concatenate([res.results[c]["out_acc"] for c in range(CORES)], 0)
    out = full[perm_n]
    return out.astype(np.float32), res


def kernel(poi_embs, src_row, src_col, src_val, tar_row, tar_col, tar_val,
           num_pois, num_hyperedges, **_ignored):
    out, _ = _run(
        np.asarray(poi_embs, np.float32),
        np.asarray(src_row).astype(np.int64),
        np.asarray(src_col).astype(np.int64),
        np.asarray(src_val, np.float32),
        np.asarray(tar_row).astype(np.int64),
        np.asarray(tar_col).astype(np.int64),
        np.asarray(tar_val, np.float32),
        int(num_pois), int(num_hyperedges),
        use_bf16=False,
    )
    return out


# revision 6
# speedup vs baseline: 2.5537x; 2.5537x over previous
"""Trainium2 Bass kernel for nn_DirectedHyperConvNetwork (3-layer hyper-graph
message passing: 6 COO SpMMs + residual + dropout + mean).

Strategy (8 NeuronCores, SPMD, one NEFF):
 - Each SpMM's destination rows are dealt degree-balanced into 8*T tiles of
   128 rows (host-side free row permutation; all index spaces remapped).
 - Per tile: bulk `dma_gather` pulls the source rows for ~C*128 edges into
   SBUF (edge i -> partition i%128, chunk i//128). Tables > 32767 rows are
   split at HALF for the int16 index limit (two gathers per tile).
 - Segment-sum on the PE: per 128-edge chunk, DVE builds
   W[p, r] = val[p] * (row[p] == r) with one fused tensor_scalar, and the PE
   accumulates W.T @ G into a PSUM tile over all chunks.
 - msg/x tables are republished between phases with an 8-core AllGather.
 - Dropout masks are bit-exact jax-CPU threefry, computed host-side (they
   depend only on the fixed key/shape) and passed as inputs.
 - Output mean accumulates into the (zero-initialized) output via SWDGE
   accumulate-DMA of 0.25-scaled tiles.
"""

import os
import subprocess
import sys
import tempfile

import numpy as np

P = 128
D = 256
CORES = 8
NUM_LAYERS = 3
KEEP = 0.9

# real problem dims
N_POIS = 50000
N_HYPER = 50000
NNZ = 1600000
HALF_DEFAULT = 32768


# --------------------------------------------------------------------------
# host-side preprocessing
# --------------------------------------------------------------------------

def _snake_pack(dest, n_rows, n_bins):
    """Deal rows of one destination space into n_bins bins of 128 rows,
    balancing edge counts. Returns perm[row] -> global padded position
    (bin*128 + round)."""
    deg = np.bincount(dest, minlength=n_rows)
    order = np.argsort(-deg, kind="stable")
    padded = np.full(n_bins * P, -1, np.int64)
    padded[:n_rows] = order
    rounds = padded.reshape(P, n_bins).copy()
    rounds[1::2] = rounds[1::2, ::-1]          # snake
    perm = np.empty(n_rows, np.int64)
    rr, bb = np.meshgrid(np.arange(P), np.arange(n_bins), indexing="ij")
    valid = rounds >= 0
    perm[rounds[valid]] = (bb * P + rr)[valid]
    return perm


def _layout_edges(dest_g, col_g, val, n_bins, half):
    """Assign edges to (bin, chunk, partition) slots.

    dest_g: per-edge global padded destination position.
    col_g: per-edge global padded gather index (into the permuted table).
    Returns (idxlo_w, idxhi_w, row_s, val_s, C_LO, C_HI):
      idxlo_w [128, n_bins, 8*C_LO] int16   (16-wrapped, 8x replicated)
      idxhi_w [128, n_bins, 8*C_HI] int16
      row_s   [128, n_bins, C] f32          (C = C_LO + C_HI)
      val_s   [128, n_bins, C] f32
    """
    bin_e = dest_g // P
    part_r = (dest_g % P).astype(np.float64)
    is_hi = col_g >= half
    key = bin_e * 2 + is_hi
    order = np.argsort(key, kind="stable")
    key_s = key[order]
    counts = np.bincount(key_s, minlength=n_bins * 2)
    starts = np.concatenate([[0], np.cumsum(counts)[:-1]])
    rank = np.arange(len(order)) - starts[key_s]       # rank within segment

    lo_cnt = counts[0::2]
    hi_cnt = counts[1::2]
    c_lo = max(1, int(np.ceil(lo_cnt.max() / P)))
    c_hi = max(1, int(np.ceil(hi_cnt.max() / P)))
    c_tot = c_lo + c_hi

    idxlo_flat = np.zeros((n_bins, c_lo * P), np.int16)
    idxhi_flat = np.zeros((n_bins, c_hi * P), np.int16)
    row_cv = np.zeros((n_bins, c_tot, P), np.float32)
    val_cv = np.zeros((n_bins, c_tot, P), np.float32)

    e = order
    ki = key_s
    lo_m = (ki % 2) == 0
    b_lo = ki[lo_m] // 2
    r_lo = rank[lo_m]
    idxlo_flat[b_lo, r_lo] = col_g[e[lo_m]].astype(np.int16)
    row_cv[b_lo, r_lo // P, r_lo % P] = part_r[e[lo_m]]
    val_cv[b_lo, r_lo // P, r_lo % P] = val[e[lo_m]]

    hi_m = ~lo_m
    b_hi = ki[hi_m] // 2
    r_hi = rank[hi_m]
    idxhi_flat[b_hi, r_hi] = (col_g[e[hi_m]] - half).astype(np.int16)
    row_cv[b_hi, c_lo + r_hi // P, r_hi % P] = part_r[e[hi_m]]
    val_cv[b_hi, c_lo + r_hi // P, r_hi % P] = val[e[hi_m]]

    def wrap(flat, c):
        # [n_bins, c*128] -> [128, n_bins, 8c]: element i of bin b at
        # [i%16, b, i//16], replicated over the 8 16-partition groups
        a = flat.reshape(n_bins, c * 8, 16).transpose(2, 0, 1)
        return np.tile(a, (8, 1, 1))

    idxlo_w = wrap(idxlo_flat, c_lo)
    idxhi_w = wrap(idxhi_flat, c_hi)
    row_s = row_cv.transpose(2, 0, 1).astype(np.float32)
    val_s = val_cv.transpose(2, 0, 1).astype(np.float32)
    return idxlo_w, idxhi_w, row_s, val_s, c_lo, c_hi


_MASK_CODE = """
import numpy as np, jax
import sys
n, d, layers = int(sys.argv[1]), int(sys.argv[2]), int(sys.argv[3])
key = jax.random.key(42)
out = np.empty((layers, n, d), np.bool_)
for i in range(layers):
    out[i] = np.asarray(jax.random.bernoulli(jax.random.fold_in(key, i), 0.9, (n, d)))
np.save(sys.argv[4], np.packbits(out, axis=-1))
"""


def _compute_masks(n, d, layers):
    """Dropout masks, bit-exact with the reference (jax threefry on CPU)."""
    with tempfile.TemporaryDirectory() as td:
        path = os.path.join(td, "m.npy")
        env = dict(os.environ)
        env["JAX_PLATFORMS"] = "cpu"
        subprocess.run(
            [sys.executable, "-c", _MASK_CODE, str(n), str(d), str(layers), path],
            check=True, env=env, capture_output=True,
        )
        packed = np.load(path)
    return np.unpackbits(packed, axis=-1).astype(bool)[:, :, :d]


# --------------------------------------------------------------------------
# device kernel builder
# --------------------------------------------------------------------------

def _build_nc(T, ct_lo, ct_hi, cs_lo, cs_hi, half, gtot, use_bf16):
    import concourse.bacc as bacc
    import concourse.mybir as mybir
    import concourse.tile as tile

    f32 = mybir.dt.float32
    gdt = mybir.dt.bfloat16 if use_bf16 else f32
    i16 = mybir.dt.int16
    RPC = T * P
    ct = ct_lo + ct_hi
    cs = cs_lo + cs_hi

    nc = bacc.Bacc("TRN2", target_bir_lowering=False, debug=False,
                   num_devices=CORES, num_swdge_queues=4)

    def din(name, shape, dt):
        return nc.dram_tensor(name, shape, dt, kind="ExternalInput")

    x0_full = din("x0_full", [gtot, D], gdt)
    x0_slice = din("x0_slice", [RPC, D], f32)
    masks_d = din("masks", [NUM_LAYERS, RPC, D], f32)
    cmax = max(ct, cs)
    iota_d = din("iota", [P, cmax * P], gdt)
    it_lo = din("it_lo", [P, T * 8 * ct_lo], i16)
    it_hi = din("it_hi", [P, T * 8 * ct_hi], i16)
    rt_d = din("rt", [P, T * ct], gdt)
    vt_d = din("vt", [P, T * ct], gdt)
    is_lo = din("is_lo", [P, T * 8 * cs_lo], i16)
    is_hi = din("is_hi", [P, T * 8 * cs_hi], i16)
    rs_d = din("rs", [P, T * cs], gdt)
    vs_d = din("vs", [P, T * cs], gdt)

    out_acc = nc.dram_tensor("out_acc", [RPC, D], f32, kind="ExternalOutput")

    msg_loc = nc.dram_tensor("msg_loc", [RPC, D], gdt)
    msg_full = nc.dram_tensor("msg_full", [gtot, D], gdt, addr_space="Shared")
    xl_loc = nc.dram_tensor("xl_loc", [RPC, D], f32)
    x_full = nc.dram_tensor("x_full", [gtot, D], gdt, addr_space="Shared")
    if use_bf16:
        xl_locb = nc.dram_tensor("xl_locb", [RPC, D], gdt)
    else:
        xl_locb = xl_loc

    rg = [list(range(CORES))]

    with tile.TileContext(nc) as tc, \
         tc.tile_pool(name="res", bufs=1) as res, \
         tc.tile_pool(name="gp", bufs=2) as gp, \
         tc.tile_pool(name="wp", bufs=2) as wp, \
         tc.tile_pool(name="sm", bufs=3) as sm, \
         tc.tile_pool(name="pp", bufs=4, space="PSUM") as pp:

        # resident SBUF data
        iota_sb = res.tile([P, cmax * P], gdt)
        nc.sync.dma_start(iota_sb[:], iota_d[:, :])
        ed = {}
        for tag, (ilo, ihi, rr, vv, clo, chi, c) in {
            "t": (it_lo, it_hi, rt_d, vt_d, ct_lo, ct_hi, ct),
            "s": (is_lo, is_hi, rs_d, vs_d, cs_lo, cs_hi, cs),
        }.items():
            ilo_sb = res.tile([P, T * 8 * clo], i16, tag=f"ilo{tag}")
            ihi_sb = res.tile([P, T * 8 * chi], i16, tag=f"ihi{tag}")
            r_sb = res.tile([P, T * c], gdt, tag=f"r{tag}")
            v_sb = res.tile([P, T * c], gdt, tag=f"v{tag}")
            nc.sync.dma_start(ilo_sb[:], ilo[:, :])
            nc.sync.dma_start(ihi_sb[:], ihi[:, :])
            nc.sync.dma_start(r_sb[:], rr[:, :])
            nc.sync.dma_start(v_sb[:], vv[:, :])
            ed[tag] = (ilo_sb, ihi_sb, r_sb, v_sb, clo, chi, c)

        MAXC = 8     # max chunks (1024 indices) per dma_gather instruction
        qn = [0]     # SWDGE queue rotation

        def spmm_tile(tag, table, t, post):
            ilo_sb, ihi_sb, r_sb, v_sb, clo, chi, c = ed[tag]
            g = gp.tile([P, c, D], gdt, tag="g")
            for base, cnt, tab, idx_sb in (
                (0, clo, table[:half, :], ilo_sb),
                (clo, chi, table[half:, :], ihi_sb),
            ):
                cn = cnt
                for c0 in range(0, cn, MAXC):
                    cw = min(MAXC, cn - c0)
                    nc.gpsimd.dma_gather(
                        out_ap=g[:, base + c0:base + c0 + cw, :],
                        in_ap=tab,
                        idxs_ap=idx_sb[:, t * 8 * cn + 8 * c0:
                                       t * 8 * cn + 8 * (c0 + cw)],
                        num_idxs=cw * P,
                        num_idxs_reg=cw * P,
                        elem_size=D,
                        queue_num=qn[0] % 4,
                    )
                    qn[0] += 1
            # batched indicator build: W[p, k, r] = val[p,k]*(iota[r]==row[p,k])
            w = wp.tile([P, c, P], gdt, tag="w")
            nc.vector.tensor_tensor(
                out=w[:, :, :],
                in0=iota_sb[:, :c * P].rearrange("p (c r) -> p c r", r=P),
                in1=r_sb[:, t * c:(t + 1) * c].to_broadcast((P, c, P)),
                op=mybir.AluOpType.is_equal,
            )
            nc.vector.tensor_tensor(
                out=w[:, :, :],
                in0=w[:, :, :],
                in1=v_sb[:, t * c:(t + 1) * c].to_broadcast((P, c, P)),
                op=mybir.AluOpType.mult,
            )
            ps = pp.tile([P, D], f32, space="PSUM", tag="ps")
            for k in range(c):
                nc.tensor.matmul(
                    out=ps[:],
                    lhsT=w[:, k, :],
                    rhs=g[:, k, :],
                    start=(k == 0),
                    stop=(k == c - 1),
                )
            post(t, ps)

        for l in range(NUM_LAYERS):
            xtab = x0_full if l == 0 else x_full

            def post_tar(t, ps):
                msb = sm.tile([P, D], gdt, tag="msb")
                nc.vector.tensor_copy(msb[:], ps[:])
                nc.sync.dma_start(msg_loc[t * P:(t + 1) * P, :], msb[:])

            for t in range(T):
                spmm_tile("t", xtab, t, post_tar)

            nc.gpsimd.collective_compute(
                "AllGather", mybir.AluOpType.bypass, replica_groups=rg,
                ins=[msg_loc.ap().opt()], outs=[msg_full.ap().opt()],
            )

            def post_src(t, ps, l=l):
                xprev = sm.tile([P, D], f32, tag="xprev")
                src_prev = x0_slice if l == 0 else xl_loc
                nc.sync.dma_start(xprev[:], src_prev[t * P:(t + 1) * P, :])
                mk = sm.tile([P, D], f32, tag="mk")
                nc.sync.dma_start(mk[:], masks_d[l, t * P:(t + 1) * P, :])
                xn = sm.tile([P, D], f32, tag="xn")
                nc.vector.tensor_tensor(out=xn[:], in0=ps[:], in1=xprev[:],
                                        op=mybir.AluOpType.add)
                nc.vector.tensor_tensor(out=xn[:], in0=xn[:], in1=mk[:],
                                        op=mybir.AluOpType.mult)
                xq = sm.tile([P, D], f32, tag="xq")
                nc.vector.tensor_scalar_mul(xq[:], xn[:], 0.25)
                nc.gpsimd.dma_start(out=out_acc[t * P:(t + 1) * P, :],
                                    in_=xq[:], accum_op=mybir.AluOpType.add)
                if l == 0:
                    xq0 = sm.tile([P, D], f32, tag="xq0")
                    nc.vector.tensor_scalar_mul(xq0[:], xprev[:], 0.25)
                    nc.gpsimd.dma_start(out=out_acc[t * P:(t + 1) * P, :],
                                        in_=xq0[:],
                                        accum_op=mybir.AluOpType.add)
                if l < NUM_LAYERS - 1:
                    nc.sync.dma_start(xl_loc[t * P:(t + 1) * P, :], xn[:])
                    if use_bf16:
                        xnb = sm.tile([P, D], gdt, tag="xnb")
                        nc.vector.tensor_copy(xnb[:], xn[:])
                        nc.sync.dma_start(xl_locb[t * P:(t + 1) * P, :],
                                          xnb[:])

            for t in range(T):
                spmm_tile("s", msg_full, t, post_src)

            if l < NUM_LAYERS - 1:
                nc.gpsimd.collective_compute(
                    "AllGather", mybir.AluOpType.bypass, replica_groups=rg,
                    ins=[xl_locb.ap().opt()], outs=[x_full.ap().opt()],
                )

    nc.compile()
    return nc


# --------------------------------------------------------------------------
# public entry point
# --------------------------------------------------------------------------

def _run(poi_embs, src_row, src_col, src_val, tar_row, tar_col, tar_val,
         n_pois, n_hyper, use_bf16=True, trace=False):
    from concourse.bass_utils import run_bass_kernel_spmd

    n_bins_n = -(-n_pois // P)
    n_bins_n = -(-n_bins_n // CORES) * CORES     # multiple of CORES
    n_bins_h = -(-n_hyper // P)
    n_bins_h = -(-n_bins_h // CORES) * CORES
    n_bins = max(n_bins_n, n_bins_h)             # same T for both phases
    T = n_bins // CORES
    RPC = T * P
    gtot = n_bins * P
    half = min(HALF_DEFAULT, (gtot // 2 + 255) & ~255)

    perm_n = _snake_pack(src_row, n_pois, n_bins)    # POI space
    perm_h = _snake_pack(tar_row, n_hyper, n_bins)   # hyperedge space

    # tar-SpMM: dest in H-space, gathers from POI table
    it_lo, it_hi, rt, vt, ct_lo, ct_hi = _layout_edges(
        perm_h[tar_row], perm_n[tar_col], tar_val, n_bins, half)
    # src-SpMM: dest in N-space, gathers from hyperedge (msg) table
    is_lo, is_hi, rs, vs, cs_lo, cs_hi = _layout_edges(
        perm_n[src_row], perm_h[src_col], src_val, n_bins, half)

    x0_full = np.zeros((gtot, D), np.float32)
    x0_full[perm_n] = poi_embs
    masks = _compute_masks(n_pois, D, NUM_LAYERS)
    mask_scaled = np.zeros((NUM_LAYERS, gtot, D), np.float32)
    mask_scaled[:, perm_n] = masks.astype(np.float32) * np.float32(1.0 / KEEP)

    cmax = max(ct_lo + ct_hi, cs_lo + cs_hi)
    iota = np.broadcast_to(
        np.tile(np.arange(P, dtype=np.float32), cmax), (P, cmax * P)).copy()

    if use_bf16:
        import ml_dtypes
        bdt = ml_dtypes.bfloat16
        x0_tab = x0_full.astype(bdt)
        iota = iota.astype(bdt)
        rt, vt, rs, vs = (a.astype(bdt) for a in (rt, vt, rs, vs))
    else:
        x0_tab = x0_full

    in_maps = []
    for c in range(CORES):
        bs = slice(c * T, (c + 1) * T)
        rows = slice(c * RPC, (c + 1) * RPC)
        in_maps.append({
            "x0_full": x0_tab,
            "x0_slice": x0_full[rows],
            "masks": np.ascontiguousarray(mask_scaled[:, rows]),
            "iota": iota,
            "it_lo": np.ascontiguousarray(it_lo[:, bs]).reshape(P, -1),
            "it_hi": np.ascontiguousarray(it_hi[:, bs]).reshape(P, -1),
            "rt": np.ascontiguousarray(rt[:, bs]).reshape(P, -1),
            "vt": np.ascontiguousarray(vt[:, bs]).reshape(P, -1),
            "is_lo": np.ascontiguousarray(is_lo[:, bs]).reshape(P, -1),
            "is_hi": np.ascontiguousarray(is_hi[:, bs]).reshape(P, -1),
            "rs": np.ascontiguousarray(rs[:, bs]).reshape(P, -1),
            "vs": np.ascontiguousarray(vs[:, bs]).reshape(P, -1),
        })

    nc = _build_nc(T, ct_lo, ct_hi, cs_lo, cs_hi, half, gtot, use_bf16)
    kw = {"trace": True} if trace else {}
    res = run_bass_kernel_spmd(nc, in_maps, core_ids=list(range(CORES)), **kw)

    full = np.concatenate([res.results[c]["out_acc"] for c in range(CORES)], 0)
    out = full[perm_n]
    return out.astype(np.float32), res


def kernel(poi_embs, src_row, src_col, src_val, tar_row, tar_col, tar_val,
           num_pois, num_hyperedges, **_ignored):
    out, _ = _run(
        np.asarray(poi_embs, np.float32),
        np.asarray(src_row).astype(np.int64),
        np.asarray(src_col).astype(np.int64),
        np.asarray(src_val, np.float32),
        np.asarray(tar_row).astype(np.int64),
        np.asarray(tar_col).astype(np.int64),
        np.asarray(tar_val, np.float32),
        int(num_pois), int(num_hyperedges),
        use_bf16=False,
    )
    return out


# revision 8
# speedup vs baseline: 2.5739x; 1.0079x over previous
"""Trainium2 Bass kernel for nn_DirectedHyperConvNetwork (3-layer hyper-graph
message passing: 6 COO SpMMs + residual + dropout + mean).

Strategy (8 NeuronCores, SPMD, one NEFF):
 - Each SpMM's destination rows are dealt degree-balanced into 8*T tiles of
   128 rows (host-side free row permutation; all index spaces remapped).
 - Per tile: bulk `dma_gather` pulls the source rows for ~C*128 edges into
   SBUF (edge i -> partition i%128, chunk i//128). Tables > 32767 rows are
   split at HALF for the int16 index limit (two gathers per tile).
 - Segment-sum on the PE: per 128-edge chunk, DVE builds
   W[p, r] = val[p] * (row[p] == r) with one fused tensor_scalar, and the PE
   accumulates W.T @ G into a PSUM tile over all chunks.
 - msg/x tables are republished between phases with an 8-core AllGather.
 - Dropout masks are bit-exact jax-CPU threefry, computed host-side (they
   depend only on the fixed key/shape) and passed as inputs.
 - Output mean accumulates into the (zero-initialized) output via SWDGE
   accumulate-DMA of 0.25-scaled tiles.
"""

import os
import subprocess
import sys
import tempfile

import numpy as np

P = 128
D = 256
CORES = 8
NUM_LAYERS = 3
KEEP = 0.9

# real problem dims
N_POIS = 50000
N_HYPER = 50000
NNZ = 1600000
HALF_DEFAULT = 32768


# --------------------------------------------------------------------------
# host-side preprocessing
# --------------------------------------------------------------------------

def _snake_pack(dest, n_rows, n_bins):
    """Deal rows of one destination space into n_bins bins of 128 rows,
    balancing edge counts. Returns perm[row] -> global padded position
    (bin*128 + round)."""
    deg = np.bincount(dest, minlength=n_rows)
    order = np.argsort(-deg, kind="stable")
    padded = np.full(n_bins * P, -1, np.int64)
    padded[:n_rows] = order
    rounds = padded.reshape(P, n_bins).copy()
    rounds[1::2] = rounds[1::2, ::-1]          # snake
    perm = np.empty(n_rows, np.int64)
    rr, bb = np.meshgrid(np.arange(P), np.arange(n_bins), indexing="ij")
    valid = rounds >= 0
    perm[rounds[valid]] = (bb * P + rr)[valid]
    return perm


def _layout_edges(dest_g, col_g, val, n_bins, half):
    """Assign edges to (bin, chunk, partition) slots.

    dest_g: per-edge global padded destination position.
    col_g: per-edge global padded gather index (into the permuted table).
    Returns (idxlo_w, idxhi_w, row_s, val_s, C_LO, C_HI):
      idxlo_w [128, n_bins, 8*C_LO] int16   (16-wrapped, 8x replicated)
      idxhi_w [128, n_bins, 8*C_HI] int16
      row_s   [128, n_bins, C] f32          (C = C_LO + C_HI)
      val_s   [128, n_bins, C] f32
    """
    bin_e = dest_g // P
    part_r = (dest_g % P).astype(np.float64)
    is_hi = col_g >= half
    key = bin_e * 2 + is_hi
    order = np.argsort(key, kind="stable")
    key_s = key[order]
    counts = np.bincount(key_s, minlength=n_bins * 2)
    starts = np.concatenate([[0], np.cumsum(counts)[:-1]])
    rank = np.arange(len(order)) - starts[key_s]       # rank within segment

    lo_cnt = counts[0::2]
    hi_cnt = counts[1::2]
    c_lo = max(1, int(np.ceil(lo_cnt.max() / P)))
    c_hi = max(1, int(np.ceil(hi_cnt.max() / P)))
    c_tot = c_lo + c_hi

    idxlo_flat = np.zeros((n_bins, c_lo * P), np.int16)
    idxhi_flat = np.zeros((n_bins, c_hi * P), np.int16)
    row_cv = np.zeros((n_bins, c_tot, P), np.float32)
    val_cv = np.zeros((n_bins, c_tot, P), np.float32)

    e = order
    ki = key_s
    lo_m = (ki % 2) == 0
    b_lo = ki[lo_m] // 2
    r_lo = rank[lo_m]
    idxlo_flat[b_lo, r_lo] = col_g[e[lo_m]].astype(np.int16)
    row_cv[b_lo, r_lo // P, r_lo % P] = part_r[e[lo_m]]
    val_cv[b_lo, r_lo // P, r_lo % P] = val[e[lo_m]]

    hi_m = ~lo_m
    b_hi = ki[hi_m] // 2
    r_hi = rank[hi_m]
    idxhi_flat[b_hi, r_hi] = (col_g[e[hi_m]] - half).astype(np.int16)
    row_cv[b_hi, c_lo + r_hi // P, r_hi % P] = part_r[e[hi_m]]
    val_cv[b_hi, c_lo + r_hi // P, r_hi % P] = val[e[hi_m]]

    def wrap(flat, c):
        # [n_bins, c*128] -> [128, n_bins, 8c]: element i of bin b at
        # [i%16, b, i//16], replicated over the 8 16-partition groups
        a = flat.reshape(n_bins, c * 8, 16).transpose(2, 0, 1)
        return np.tile(a, (8, 1, 1))

    idxlo_w = wrap(idxlo_flat, c_lo)
    idxhi_w = wrap(idxhi_flat, c_hi)
    row_s = row_cv.transpose(2, 0, 1).astype(np.float32)
    val_s = val_cv.transpose(2, 0, 1).astype(np.float32)
    return idxlo_w, idxhi_w, row_s, val_s, c_lo, c_hi


_MASK_CODE = """
import numpy as np, jax
import sys
n, d, layers = int(sys.argv[1]), int(sys.argv[2]), int(sys.argv[3])
key = jax.random.key(42)
out = np.empty((layers, n, d), np.bool_)
for i in range(layers):
    out[i] = np.asarray(jax.random.bernoulli(jax.random.fold_in(key, i), 0.9, (n, d)))
np.save(sys.argv[4], np.packbits(out, axis=-1))
"""


def _compute_masks(n, d, layers):
    """Dropout masks, bit-exact with the reference (jax threefry on CPU)."""
    with tempfile.TemporaryDirectory() as td:
        path = os.path.join(td, "m.npy")
        env = dict(os.environ)
        env["JAX_PLATFORMS"] = "cpu"
        subprocess.run(
            [sys.executable, "-c", _MASK_CODE, str(n), str(d), str(layers), path],
            check=True, env=env, capture_output=True,
        )
        packed = np.load(path)
    return np.unpackbits(packed, axis=-1).astype(bool)[:, :, :d]


# --------------------------------------------------------------------------
# device kernel builder
# --------------------------------------------------------------------------

def _build_nc(T, ct_lo, ct_hi, cs_lo, cs_hi, half, gtot, use_bf16):
    import concourse.bacc as bacc
    import concourse.mybir as mybir
    import concourse.tile as tile

    f32 = mybir.dt.float32
    gdt = mybir.dt.bfloat16 if use_bf16 else f32
    i16 = mybir.dt.int16
    RPC = T * P
    ct = ct_lo + ct_hi
    cs = cs_lo + cs_hi

    nc = bacc.Bacc("TRN2", target_bir_lowering=False, debug=False,
                   num_devices=CORES, num_swdge_queues=4)

    def din(name, shape, dt):
        return nc.dram_tensor(name, shape, dt, kind="ExternalInput")

    x0_full = din("x0_full", [gtot, D], gdt)
    x0_slice = din("x0_slice", [RPC, D], f32)
    masks_d = din("masks", [NUM_LAYERS, RPC, D], f32)
    cmax = max(ct, cs)
    iota_d = din("iota", [P, cmax * P], gdt)
    it_lo = din("it_lo", [P, T * 8 * ct_lo], i16)
    it_hi = din("it_hi", [P, T * 8 * ct_hi], i16)
    rt_d = din("rt", [P, T * ct], gdt)
    vt_d = din("vt", [P, T * ct], gdt)
    is_lo = din("is_lo", [P, T * 8 * cs_lo], i16)
    is_hi = din("is_hi", [P, T * 8 * cs_hi], i16)
    rs_d = din("rs", [P, T * cs], gdt)
    vs_d = din("vs", [P, T * cs], gdt)

    out_acc = nc.dram_tensor("out_acc", [RPC, D], f32, kind="ExternalOutput")

    msg_loc = nc.dram_tensor("msg_loc", [RPC, D], gdt)
    msg_full = nc.dram_tensor("msg_full", [gtot, D], gdt, addr_space="Shared")
    xl_loc = nc.dram_tensor("xl_loc", [RPC, D], f32)
    x_full = nc.dram_tensor("x_full", [gtot, D], gdt, addr_space="Shared")
    if use_bf16:
        xl_locb = nc.dram_tensor("xl_locb", [RPC, D], gdt)
    else:
        xl_locb = xl_loc

    rg = [list(range(CORES))]

    with tile.TileContext(nc) as tc, \
         tc.tile_pool(name="res", bufs=1) as res, \
         tc.tile_pool(name="gp", bufs=2) as gp, \
         tc.tile_pool(name="wp", bufs=2) as wp, \
         tc.tile_pool(name="sm", bufs=3) as sm, \
         tc.tile_pool(name="pp", bufs=4, space="PSUM") as pp:

        # resident SBUF data
        iota_sb = res.tile([P, cmax * P], gdt)
        nc.sync.dma_start(iota_sb[:], iota_d[:, :])
        ed = {}
        for tag, (ilo, ihi, rr, vv, clo, chi, c) in {
            "t": (it_lo, it_hi, rt_d, vt_d, ct_lo, ct_hi, ct),
            "s": (is_lo, is_hi, rs_d, vs_d, cs_lo, cs_hi, cs),
        }.items():
            ilo_sb = res.tile([P, T * 8 * clo], i16, tag=f"ilo{tag}")
            ihi_sb = res.tile([P, T * 8 * chi], i16, tag=f"ihi{tag}")
            r_sb = res.tile([P, T * c], gdt, tag=f"r{tag}")
            v_sb = res.tile([P, T * c], gdt, tag=f"v{tag}")
            nc.sync.dma_start(ilo_sb[:], ilo[:, :])
            nc.sync.dma_start(ihi_sb[:], ihi[:, :])
            nc.sync.dma_start(r_sb[:], rr[:, :])
            nc.sync.dma_start(v_sb[:], vv[:, :])
            ed[tag] = (ilo_sb, ihi_sb, r_sb, v_sb, clo, chi, c)

        MAXC = 8     # max chunks (1024 indices) per dma_gather instruction
        qn = [0]     # SWDGE queue rotation

        def spmm_tile(tag, table, t, post):
            ilo_sb, ihi_sb, r_sb, v_sb, clo, chi, c = ed[tag]
            g = gp.tile([P, c, D], gdt, tag="g")
            for base, cnt, tab, idx_sb in (
                (0, clo, table[:half, :], ilo_sb),
                (clo, chi, table[half:, :], ihi_sb),
            ):
                cn = cnt
                for c0 in range(0, cn, MAXC):
                    cw = min(MAXC, cn - c0)
                    nc.gpsimd.dma_gather(
                        out_ap=g[:, base + c0:base + c0 + cw, :],
                        in_ap=tab,
                        idxs_ap=idx_sb[:, t * 8 * cn + 8 * c0:
                                       t * 8 * cn + 8 * (c0 + cw)],
                        num_idxs=cw * P,
                        num_idxs_reg=cw * P,
                        elem_size=D,
                        queue_num=qn[0] % 4,
                    )
                    qn[0] += 1
            # batched indicator build: W[p, k, r] = val[p,k]*(iota[r]==row[p,k])
            w = wp.tile([P, c, P], gdt, tag="w")
            nc.vector.tensor_tensor(
                out=w[:, :, :],
                in0=iota_sb[:, :c * P].rearrange("p (c r) -> p c r", r=P),
                in1=r_sb[:, t * c:(t + 1) * c].to_broadcast((P, c, P)),
                op=mybir.AluOpType.is_equal,
            )
            nc.vector.tensor_tensor(
                out=w[:, :, :],
                in0=w[:, :, :],
                in1=v_sb[:, t * c:(t + 1) * c].to_broadcast((P, c, P)),
                op=mybir.AluOpType.mult,
            )
            ps = pp.tile([P, D], f32, space="PSUM", tag="ps")
            for k in range(c):
                nc.tensor.matmul(
                    out=ps[:],
                    lhsT=w[:, k, :],
                    rhs=g[:, k, :],
                    start=(k == 0),
                    stop=(k == c - 1),
                )
            post(t, ps)

        for l in range(NUM_LAYERS):
            xtab = x0_full if l == 0 else x_full

            def post_tar(t, ps):
                msb = sm.tile([P, D], gdt, tag="msb")
                nc.scalar.copy(msb[:], ps[:])
                nc.sync.dma_start(msg_loc[t * P:(t + 1) * P, :], msb[:])

            for t in range(T):
                spmm_tile("t", xtab, t, post_tar)

            nc.gpsimd.collective_compute(
                "AllGather", mybir.AluOpType.bypass, replica_groups=rg,
                ins=[msg_loc.ap().opt()], outs=[msg_full.ap().opt()],
            )

            def post_src(t, ps, l=l):
                xprev = sm.tile([P, D], f32, tag="xprev")
                src_prev = x0_slice if l == 0 else xl_loc
                nc.sync.dma_start(xprev[:], src_prev[t * P:(t + 1) * P, :])
                mk = sm.tile([P, D], f32, tag="mk")
                nc.sync.dma_start(mk[:], masks_d[l, t * P:(t + 1) * P, :])
                xn = sm.tile([P, D], f32, tag="xn")
                nc.vector.tensor_tensor(out=xn[:], in0=ps[:], in1=xprev[:],
                                        op=mybir.AluOpType.add)
                nc.vector.tensor_tensor(out=xn[:], in0=xn[:], in1=mk[:],
                                        op=mybir.AluOpType.mult)
                xq = sm.tile([P, D], f32, tag="xq")
                nc.scalar.mul(xq[:], xn[:], 0.25)
                nc.gpsimd.dma_start(out=out_acc[t * P:(t + 1) * P, :],
                                    in_=xq[:], accum_op=mybir.AluOpType.add)
                if l == 0:
                    xq0 = sm.tile([P, D], f32, tag="xq0")
                    nc.scalar.mul(xq0[:], xprev[:], 0.25)
                    nc.gpsimd.dma_start(out=out_acc[t * P:(t + 1) * P, :],
                                        in_=xq0[:],
                                        accum_op=mybir.AluOpType.add)
                if l < NUM_LAYERS - 1:
                    nc.sync.dma_start(xl_loc[t * P:(t + 1) * P, :], xn[:])
                    if use_bf16:
                        xnb = sm.tile([P, D], gdt, tag="xnb")
                        nc.scalar.copy(xnb[:], xn[:])
                        nc.sync.dma_start(xl_locb[t * P:(t + 1) * P, :],
                                          xnb[:])

            for t in range(T):
                spmm_tile("s", msg_full, t, post_src)

            if l < NUM_LAYERS - 1:
                nc.gpsimd.collective_compute(
                    "AllGather", mybir.AluOpType.bypass, replica_groups=rg,
                    ins=[xl_locb.ap().opt()], outs=[x_full.ap().opt()],
                )

    nc.compile()
    return nc


# --------------------------------------------------------------------------
# public entry point
# --------------------------------------------------------------------------

def _run(poi_embs, src_row, src_col, src_val, tar_row, tar_col, tar_val,
         n_pois, n_hyper, use_bf16=True, trace=False):
    from concourse.bass_utils import run_bass_kernel_spmd

    n_bins_n = -(-n_pois // P)
    n_bins_n = -(-n_bins_n // CORES) * CORES     # multiple of CORES
    n_bins_h = -(-n_hyper // P)
    n_bins_h = -(-n_bins_h // CORES) * CORES
    n_bins = max(n_bins_n, n_bins_h)             # same T for both phases
    T = n_bins // CORES
    RPC = T * P
    gtot = n_bins * P
    half = min(HALF_DEFAULT, (gtot // 2 + 255) & ~255)

    perm_n = _snake_pack(src_row, n_pois, n_bins)    # POI space
    perm_h = _snake_pack(tar_row, n_hyper, n_bins)   # hyperedge space

    # tar-SpMM: dest in H-space, gathers from POI table
    it_lo, it_hi, rt, vt, ct_lo, ct_hi = _layout_edges(
        perm_h[tar_row], perm_n[tar_col], tar_val, n_bins, half)
    # src-SpMM: dest in N-space, gathers from hyperedge (msg) table
    is_lo, is_hi, rs, vs, cs_lo, cs_hi = _layout_edges(
        perm_n[src_row], perm_h[src_col], src_val, n_bins, half)

    x0_full = np.zeros((gtot, D), np.float32)
    x0_full[perm_n] = poi_embs
    masks = _compute_masks(n_pois, D, NUM_LAYERS)
    mask_scaled = np.zeros((NUM_LAYERS, gtot, D), np.float32)
    mask_scaled[:, perm_n] = masks.astype(np.float32) * np.float32(1.0 / KEEP)

    cmax = max(ct_lo + ct_hi, cs_lo + cs_hi)
    iota = np.broadcast_to(
        np.tile(np.arange(P, dtype=np.float32), cmax), (P, cmax * P)).copy()

    if use_bf16:
        import ml_dtypes
        bdt = ml_dtypes.bfloat16
        x0_tab = x0_full.astype(bdt)
        iota = iota.astype(bdt)
        rt, vt, rs, vs = (a.astype(bdt) for a in (rt, vt, rs, vs))
    else:
        x0_tab = x0_full

    in_maps = []
    for c in range(CORES):
        bs = slice(c * T, (c + 1) * T)
        rows = slice(c * RPC, (c + 1) * RPC)
        in_maps.append({
            "x0_full": x0_tab,
            "x0_slice": x0_full[rows],
            "masks": np.ascontiguousarray(mask_scaled[:, rows]),
            "iota": iota,
            "it_lo": np.ascontiguousarray(it_lo[:, bs]).reshape(P, -1),
            "it_hi": np.ascontiguousarray(it_hi[:, bs]).reshape(P, -1),
            "rt": np.ascontiguousarray(rt[:, bs]).reshape(P, -1),
            "vt": np.ascontiguousarray(vt[:, bs]).reshape(P, -1),
            "is_lo": np.ascontiguousarray(is_lo[:, bs]).reshape(P, -1),
            "is_hi": np.ascontiguousarray(is_hi[:, bs]).reshape(P, -1),
            "rs": np.ascontiguousarray(rs[:, bs]).reshape(P, -1),
            "vs": np.ascontiguousarray(vs[:, bs]).reshape(P, -1),
        })

    nc = _build_nc(T, ct_lo, ct_hi, cs_lo, cs_hi, half, gtot, use_bf16)
    kw = {"trace": True} if trace else {}
    res = run_bass_kernel_spmd(nc, in_maps, core_ids=list(range(CORES)), **kw)

    full = np.concatenate([res.results[c]["out_acc"] for c in range(CORES)], 0)
    out = full[perm_n]
    return out.astype(np.float32), res


def kernel(poi_embs, src_row, src_col, src_val, tar_row, tar_col, tar_val,
           num_pois, num_hyperedges, **_ignored):
    out, _ = _run(
        np.asarray(poi_embs, np.float32),
        np.asarray(src_row).astype(np.int64),
        np.asarray(src_col).astype(np.int64),
        np.asarray(src_val, np.float32),
        np.asarray(tar_row).astype(np.int64),
        np.asarray(tar_col).astype(np.int64),
        np.asarray(tar_val, np.float32),
        int(num_pois), int(num_hyperedges),
        use_bf16=True,
    )
    return out


# revision 12
# speedup vs baseline: 2.8413x; 1.1039x over previous
"""Trainium2 Bass kernel for nn_DirectedHyperConvNetwork (3-layer hyper-graph
message passing: 6 COO SpMMs + residual + dropout + mean).

Strategy (8 NeuronCores, SPMD, one NEFF):
 - Each SpMM's destination rows are dealt degree-balanced into 8*T tiles of
   128 rows (host-side free row permutation; all index spaces remapped).
 - Per tile: bulk `dma_gather` pulls the source rows for ~C*128 edges into
   SBUF (edge i -> partition i%128, chunk i//128). Tables > 32767 rows are
   split at HALF for the int16 index limit (two gathers per tile).
 - Segment-sum on the PE: DVE builds the per-tile indicator
   W[p, k, r] = val[p,k] * (row[p,k] == r) with two batched tensor_tensor
   ops (0-stride broadcast APs), and the PE accumulates W_k.T @ G_k into a
   PSUM tile over all chunks k (bf16 operands, fp32 accumulation).
 - msg/x tables are republished between phases with an 8-core AllGather.
 - Dropout masks are bit-exact jax-CPU threefry, computed host-side (they
   depend only on the fixed key/shape) and passed as inputs.
 - Output mean accumulates into the (zero-initialized) output via SWDGE
   accumulate-DMA of 0.25-scaled tiles.
"""

import os
import subprocess
import sys
import tempfile

import numpy as np

P = 128
D = 256
CORES = 8
NUM_LAYERS = 3
KEEP = 0.9

# real problem dims
N_POIS = 50000
N_HYPER = 50000
NNZ = 1600000
HALF_DEFAULT = 32768


# --------------------------------------------------------------------------
# host-side preprocessing
# --------------------------------------------------------------------------

def _snake_pack(dest, n_rows, n_bins):
    """Deal rows of one destination space into n_bins bins of 128 rows,
    balancing edge counts. Returns perm[row] -> global padded position
    (bin*128 + round)."""
    deg = np.bincount(dest, minlength=n_rows)
    order = np.argsort(-deg, kind="stable")
    padded = np.full(n_bins * P, -1, np.int64)
    padded[:n_rows] = order
    rounds = padded.reshape(P, n_bins).copy()
    rounds[1::2] = rounds[1::2, ::-1]          # snake
    perm = np.empty(n_rows, np.int64)
    rr, bb = np.meshgrid(np.arange(P), np.arange(n_bins), indexing="ij")
    valid = rounds >= 0
    perm[rounds[valid]] = (bb * P + rr)[valid]
    return perm


def _layout_edges(dest_g, col_g, val, n_bins, half):
    """Assign edges to (bin, chunk, partition) slots.

    dest_g: per-edge global padded destination position.
    col_g: per-edge global padded gather index (into the permuted table).
    Returns (idxlo_w, idxhi_w, row_s, val_s, C_LO, C_HI):
      idxlo_w [128, n_bins, 8*C_LO] int16   (16-wrapped, 8x replicated)
      idxhi_w [128, n_bins, 8*C_HI] int16
      row_s   [128, n_bins, C] f32          (C = C_LO + C_HI)
      val_s   [128, n_bins, C] f32
    """
    bin_e = dest_g // P
    part_r = (dest_g % P).astype(np.float64)
    is_hi = col_g >= half
    key = bin_e * 2 + is_hi
    order = np.argsort(key, kind="stable")
    key_s = key[order]
    counts = np.bincount(key_s, minlength=n_bins * 2)
    starts = np.concatenate([[0], np.cumsum(counts)[:-1]])
    rank = np.arange(len(order)) - starts[key_s]       # rank within segment

    lo_cnt = counts[0::2]
    hi_cnt = counts[1::2]
    c_lo = max(1, int(np.ceil(lo_cnt.max() / P)))
    c_hi = max(1, int(np.ceil(hi_cnt.max() / P)))
    c_tot = c_lo + c_hi

    idxlo_flat = np.zeros((n_bins, c_lo * P), np.int16)
    idxhi_flat = np.zeros((n_bins, c_hi * P), np.int16)
    row_cv = np.zeros((n_bins, c_tot, P), np.float32)
    val_cv = np.zeros((n_bins, c_tot, P), np.float32)

    e = order
    ki = key_s
    lo_m = (ki % 2) == 0
    b_lo = ki[lo_m] // 2
    r_lo = rank[lo_m]
    idxlo_flat[b_lo, r_lo] = col_g[e[lo_m]].astype(np.int16)
    row_cv[b_lo, r_lo // P, r_lo % P] = part_r[e[lo_m]]
    val_cv[b_lo, r_lo // P, r_lo % P] = val[e[lo_m]]

    hi_m = ~lo_m
    b_hi = ki[hi_m] // 2
    r_hi = rank[hi_m]
    idxhi_flat[b_hi, r_hi] = (col_g[e[hi_m]] - half).astype(np.int16)
    row_cv[b_hi, c_lo + r_hi // P, r_hi % P] = part_r[e[hi_m]]
    val_cv[b_hi, c_lo + r_hi // P, r_hi % P] = val[e[hi_m]]

    def wrap(flat, c):
        # [n_bins, c*128] -> [128, n_bins, 8c]: element i of bin b at
        # [i%16, b, i//16], replicated over the 8 16-partition groups
        a = flat.reshape(n_bins, c * 8, 16).transpose(2, 0, 1)
        return np.tile(a, (8, 1, 1))

    idxlo_w = wrap(idxlo_flat, c_lo)
    idxhi_w = wrap(idxhi_flat, c_hi)
    row_s = row_cv.transpose(2, 0, 1).astype(np.float32)
    val_s = val_cv.transpose(2, 0, 1).astype(np.float32)
    return idxlo_w, idxhi_w, row_s, val_s, c_lo, c_hi


_MASK_CODE = """
import numpy as np, jax
import sys
n, d, layers = int(sys.argv[1]), int(sys.argv[2]), int(sys.argv[3])
key = jax.random.key(42)
out = np.empty((layers, n, d), np.bool_)
for i in range(layers):
    out[i] = np.asarray(jax.random.bernoulli(jax.random.fold_in(key, i), 0.9, (n, d)))
np.save(sys.argv[4], np.packbits(out, axis=-1))
"""


def _compute_masks(n, d, layers):
    """Dropout masks, bit-exact with the reference (jax threefry on CPU)."""
    with tempfile.TemporaryDirectory() as td:
        path = os.path.join(td, "m.npy")
        env = dict(os.environ)
        env["JAX_PLATFORMS"] = "cpu"
        subprocess.run(
            [sys.executable, "-c", _MASK_CODE, str(n), str(d), str(layers), path],
            check=True, env=env, capture_output=True,
        )
        packed = np.load(path)
    return np.unpackbits(packed, axis=-1).astype(bool)[:, :, :d]


# --------------------------------------------------------------------------
# device kernel builder
# --------------------------------------------------------------------------

def _build_nc(T, TA, ct_lo, ct_hi, cs_lo, cs_hi, half, gtot, use_bf16):
    import concourse.bacc as bacc
    import concourse.mybir as mybir
    import concourse.tile as tile

    f32 = mybir.dt.float32
    gdt = mybir.dt.bfloat16 if use_bf16 else f32
    i16 = mybir.dt.int16
    RPC = T * P
    ct = ct_lo + ct_hi
    cs = cs_lo + cs_hi

    nc = bacc.Bacc("TRN2", target_bir_lowering=False, debug=False,
                   num_devices=CORES, num_swdge_queues=4)

    def din(name, shape, dt):
        return nc.dram_tensor(name, shape, dt, kind="ExternalInput")

    x0_full = din("x0_full", [gtot, D], gdt)
    x0_slice = din("x0_slice", [RPC, D], f32)
    masks_d = din("masks", [NUM_LAYERS, RPC, D], f32)
    cmax = max(ct, cs)
    iota_d = din("iota", [P, cmax * P], gdt)
    it_lo = din("it_lo", [P, T * 8 * ct_lo], i16)
    it_hi = din("it_hi", [P, T * 8 * ct_hi], i16)
    rt_d = din("rt", [P, T * ct], gdt)
    vt_d = din("vt", [P, T * ct], gdt)
    is_lo = din("is_lo", [P, T * 8 * cs_lo], i16)
    is_hi = din("is_hi", [P, T * 8 * cs_hi], i16)
    rs_d = din("rs", [P, T * cs], gdt)
    vs_d = din("vs", [P, T * cs], gdt)

    out_acc = nc.dram_tensor("out_acc", [RPC, D], f32, kind="ExternalOutput")

    msg_loc = nc.dram_tensor("msg_loc", [RPC, D], gdt)
    msg_full = nc.dram_tensor("msg_full", [gtot, D], gdt, addr_space="Shared")
    xl_loc = nc.dram_tensor("xl_loc", [RPC, D], f32)
    x_full = nc.dram_tensor("x_full", [gtot, D], gdt, addr_space="Shared")
    if use_bf16:
        xl_locb = nc.dram_tensor("xl_locb", [RPC, D], gdt)
    else:
        xl_locb = xl_loc

    rg = [list(range(CORES))]

    with tile.TileContext(nc) as tc, \
         tc.tile_pool(name="res", bufs=1) as res, \
         tc.tile_pool(name="gp", bufs=3) as gp, \
         tc.tile_pool(name="wp", bufs=2) as wp, \
         tc.tile_pool(name="sm", bufs=3) as sm, \
         tc.tile_pool(name="pp", bufs=4, space="PSUM") as pp:

        # resident SBUF data
        iota_sb = res.tile([P, cmax * P], gdt)
        nc.sync.dma_start(iota_sb[:], iota_d[:, :])
        ed = {}
        for tag, (ilo, ihi, rr, vv, clo, chi, c) in {
            "t": (it_lo, it_hi, rt_d, vt_d, ct_lo, ct_hi, ct),
            "s": (is_lo, is_hi, rs_d, vs_d, cs_lo, cs_hi, cs),
        }.items():
            ilo_sb = res.tile([P, T * 8 * clo], i16, tag=f"ilo{tag}")
            ihi_sb = res.tile([P, T * 8 * chi], i16, tag=f"ihi{tag}")
            r_sb = res.tile([P, T * c], gdt, tag=f"r{tag}")
            v_sb = res.tile([P, T * c], gdt, tag=f"v{tag}")
            nc.sync.dma_start(ilo_sb[:], ilo[:, :])
            nc.sync.dma_start(ihi_sb[:], ihi[:, :])
            nc.sync.dma_start(r_sb[:], rr[:, :])
            nc.sync.dma_start(v_sb[:], vv[:, :])
            ed[tag] = (ilo_sb, ihi_sb, r_sb, v_sb, clo, chi, c)

        MAXC = 8     # max chunks (1024 indices) per dma_gather instruction
        qn = [0]     # SWDGE queue rotation

        def spmm_tile(tag, table, t, post):
            ilo_sb, ihi_sb, r_sb, v_sb, clo, chi, c = ed[tag]
            g = gp.tile([P, c, D], gdt, tag="g")
            for base, cnt, tab, idx_sb in (
                (0, clo, table[:half, :], ilo_sb),
                (clo, chi, table[half:, :], ihi_sb),
            ):
                cn = cnt
                for c0 in range(0, cn, MAXC):
                    cw = min(MAXC, cn - c0)
                    nc.gpsimd.dma_gather(
                        out_ap=g[:, base + c0:base + c0 + cw, :],
                        in_ap=tab,
                        idxs_ap=idx_sb[:, t * 8 * cn + 8 * c0:
                                       t * 8 * cn + 8 * (c0 + cw)],
                        num_idxs=cw * P,
                        num_idxs_reg=cw * P,
                        elem_size=D,
                        queue_num=qn[0] % 4,
                    )
                    qn[0] += 1
            # batched indicator build: W[p, k, r] = val[p,k]*(iota[r]==row[p,k])
            w = wp.tile([P, c, P], gdt, tag="w")
            nc.vector.tensor_tensor(
                out=w[:, :, :],
                in0=iota_sb[:, :c * P].rearrange("p (c r) -> p c r", r=P),
                in1=r_sb[:, t * c:(t + 1) * c].to_broadcast((P, c, P)),
                op=mybir.AluOpType.is_equal,
            )
            nc.vector.tensor_tensor(
                out=w[:, :, :],
                in0=w[:, :, :],
                in1=v_sb[:, t * c:(t + 1) * c].to_broadcast((P, c, P)),
                op=mybir.AluOpType.mult,
            )
            ps = pp.tile([P, D], f32, space="PSUM", tag="ps")
            for k in range(c):
                nc.tensor.matmul(
                    out=ps[:],
                    lhsT=w[:, k, :],
                    rhs=g[:, k, :],
                    start=(k == 0),
                    stop=(k == c - 1),
                )
            post(t, ps)

        for l in range(NUM_LAYERS):
            xtab = x0_full if l == 0 else x_full

            def post_tar(t, ps):
                msb = sm.tile([P, D], gdt, tag="msb")
                nc.scalar.copy(msb[:], ps[:])
                nc.sync.dma_start(msg_loc[t * P:(t + 1) * P, :], msb[:])

            for t in range(T):
                spmm_tile("t", xtab, t, post_tar)
                if t == TA - 1:
                    nc.gpsimd.collective_compute(
                        "AllGather", mybir.AluOpType.bypass,
                        replica_groups=rg,
                        ins=[msg_loc[0:TA * P, :].opt()],
                        outs=[msg_full[0:CORES * TA * P, :].opt()],
                    )
            nc.gpsimd.collective_compute(
                "AllGather", mybir.AluOpType.bypass, replica_groups=rg,
                ins=[msg_loc[TA * P:T * P, :].opt()],
                outs=[msg_full[CORES * TA * P:CORES * T * P, :].opt()],
            )

            def post_src(t, ps, l=l):
                xprev = sm.tile([P, D], f32, tag="xprev")
                src_prev = x0_slice if l == 0 else xl_loc
                nc.sync.dma_start(xprev[:], src_prev[t * P:(t + 1) * P, :])
                mk = sm.tile([P, D], f32, tag="mk")
                nc.sync.dma_start(mk[:], masks_d[l, t * P:(t + 1) * P, :])
                xn = sm.tile([P, D], f32, tag="xn")
                nc.vector.tensor_tensor(out=xn[:], in0=ps[:], in1=xprev[:],
                                        op=mybir.AluOpType.add)
                nc.vector.tensor_tensor(out=xn[:], in0=xn[:], in1=mk[:],
                                        op=mybir.AluOpType.mult)
                xq = sm.tile([P, D], f32, tag="xq")
                nc.scalar.mul(xq[:], xn[:], 0.25)
                nc.gpsimd.dma_start(out=out_acc[t * P:(t + 1) * P, :],
                                    in_=xq[:], accum_op=mybir.AluOpType.add)
                if l == 0:
                    xq0 = sm.tile([P, D], f32, tag="xq0")
                    nc.scalar.mul(xq0[:], xprev[:], 0.25)
                    nc.gpsimd.dma_start(out=out_acc[t * P:(t + 1) * P, :],
                                        in_=xq0[:],
                                        accum_op=mybir.AluOpType.add)
                if l < NUM_LAYERS - 1:
                    nc.sync.dma_start(xl_loc[t * P:(t + 1) * P, :], xn[:])
                    if use_bf16:
                        xnb = sm.tile([P, D], gdt, tag="xnb")
                        nc.scalar.copy(xnb[:], xn[:])
                        nc.sync.dma_start(xl_locb[t * P:(t + 1) * P, :],
                                          xnb[:])

            for t in range(T):
                spmm_tile("s", msg_full, t, post_src)
                if t == TA - 1 and l < NUM_LAYERS - 1:
                    nc.gpsimd.collective_compute(
                        "AllGather", mybir.AluOpType.bypass,
                        replica_groups=rg,
                        ins=[xl_locb[0:TA * P, :].opt()],
                        outs=[x_full[0:CORES * TA * P, :].opt()],
                    )
            if l < NUM_LAYERS - 1:
                nc.gpsimd.collective_compute(
                    "AllGather", mybir.AluOpType.bypass, replica_groups=rg,
                    ins=[xl_locb[TA * P:T * P, :].opt()],
                    outs=[x_full[CORES * TA * P:CORES * T * P, :].opt()],
                )

    nc.compile()
    return nc


# --------------------------------------------------------------------------
# public entry point
# --------------------------------------------------------------------------

def _run(poi_embs, src_row, src_col, src_val, tar_row, tar_col, tar_val,
         n_pois, n_hyper, use_bf16=True, trace=False):
    from concourse.bass_utils import run_bass_kernel_spmd

    n_bins_n = -(-n_pois // P)
    n_bins_n = -(-n_bins_n // CORES) * CORES     # multiple of CORES
    n_bins_h = -(-n_hyper // P)
    n_bins_h = -(-n_bins_h // CORES) * CORES
    n_bins = max(n_bins_n, n_bins_h)             # same T for both phases
    T = n_bins // CORES
    RPC = T * P
    gtot = n_bins * P
    half = min(HALF_DEFAULT, (gtot // 2 + 255) & ~255)

    TA = (T + 1) // 2
    TB = T - TA
    # global layout supporting split AllGathers: half A = tiles [0,TA) of
    # every core (rank-major), then half B = tiles [TA,T)
    cc, tt = np.meshgrid(np.arange(CORES), np.arange(T), indexing="ij")
    bases = np.where(tt < TA, (cc * TA + tt) * P,
                     CORES * TA * P + (cc * TB + (tt - TA)) * P).reshape(-1)
    gmap = (bases[:, None] + np.arange(P)[None, :]).reshape(-1)
    l2g = gmap.reshape(CORES, T * P)          # per-core local row -> global
    perm_n = gmap[_snake_pack(src_row, n_pois, n_bins)]    # POI space
    perm_h = gmap[_snake_pack(tar_row, n_hyper, n_bins)]   # hyperedge space

    inv_gmap = np.empty_like(gmap)
    inv_gmap[gmap] = np.arange(gtot)
    packbin_n = inv_gmap[perm_n]     # bin-major position (core*T+t)*P + r
    packbin_h = inv_gmap[perm_h]
    # tar-SpMM: dest in H-space, gathers from POI table
    it_lo, it_hi, rt, vt, ct_lo, ct_hi = _layout_edges(
        packbin_h[tar_row], perm_n[tar_col], tar_val, n_bins, half)
    # src-SpMM: dest in N-space, gathers from hyperedge (msg) table
    is_lo, is_hi, rs, vs, cs_lo, cs_hi = _layout_edges(
        packbin_n[src_row], perm_h[src_col], src_val, n_bins, half)

    x0_full = np.zeros((gtot, D), np.float32)
    x0_full[perm_n] = poi_embs
    masks = _compute_masks(n_pois, D, NUM_LAYERS)
    mask_scaled = np.zeros((NUM_LAYERS, gtot, D), np.float32)
    mask_scaled[:, perm_n] = masks.astype(np.float32) * np.float32(1.0 / KEEP)

    cmax = max(ct_lo + ct_hi, cs_lo + cs_hi)
    iota = np.broadcast_to(
        np.tile(np.arange(P, dtype=np.float32), cmax), (P, cmax * P)).copy()

    if use_bf16:
        import ml_dtypes
        bdt = ml_dtypes.bfloat16
        x0_tab = x0_full.astype(bdt)
        iota = iota.astype(bdt)
        rt, vt, rs, vs = (a.astype(bdt) for a in (rt, vt, rs, vs))
    else:
        x0_tab = x0_full

    in_maps = []
    for c in range(CORES):
        bs = slice(c * T, (c + 1) * T)
        rows = l2g[c]
        in_maps.append({
            "x0_full": x0_tab,
            "x0_slice": np.ascontiguousarray(x0_full[rows]),
            "masks": np.ascontiguousarray(mask_scaled[:, rows]),
            "iota": iota,
            "it_lo": np.ascontiguousarray(it_lo[:, bs]).reshape(P, -1),
            "it_hi": np.ascontiguousarray(it_hi[:, bs]).reshape(P, -1),
            "rt": np.ascontiguousarray(rt[:, bs]).reshape(P, -1),
            "vt": np.ascontiguousarray(vt[:, bs]).reshape(P, -1),
            "is_lo": np.ascontiguousarray(is_lo[:, bs]).reshape(P, -1),
            "is_hi": np.ascontiguousarray(is_hi[:, bs]).reshape(P, -1),
            "rs": np.ascontiguousarray(rs[:, bs]).reshape(P, -1),
            "vs": np.ascontiguousarray(vs[:, bs]).reshape(P, -1),
        })

    nc = _build_nc(T, TA, ct_lo, ct_hi, cs_lo, cs_hi, half, gtot, use_bf16)
    kw = {"trace": True} if trace else {}
    res = run_bass_kernel_spmd(nc, in_maps, core_ids=list(range(CORES)), **kw)

    full = np.empty((gtot, D), np.float32)
    for c in range(CORES):
        full[l2g[c]] = res.results[c]["out_acc"]
    out = full[perm_n]
    return out.astype(np.float32), res


def kernel(poi_embs, src_row, src_col, src_val, tar_row, tar_col, tar_val,
           num_pois, num_hyperedges, **_ignored):
    out, _ = _run(
        np.asarray(poi_embs, np.float32),
        np.asarray(src_row).astype(np.int64),
        np.asarray(src_col).astype(np.int64),
        np.asarray(src_val, np.float32),
        np.asarray(tar_row).astype(np.int64),
        np.asarray(tar_col).astype(np.int64),
        np.asarray(tar_val, np.float32),
        int(num_pois), int(num_hyperedges),
        use_bf16=True,
    )
    return out


# revision 13
# speedup vs baseline: 3.0585x; 1.0765x over previous
"""Trainium2 Bass kernel for nn_DirectedHyperConvNetwork (3-layer hyper-graph
message passing: 6 COO SpMMs + residual + dropout + mean).

Strategy (8 NeuronCores, SPMD, one NEFF):
 - Each SpMM's destination rows are dealt degree-balanced into 8*T tiles of
   128 rows (host-side free row permutation; all index spaces remapped).
 - Per tile: bulk `dma_gather` pulls the source rows for ~C*128 edges into
   SBUF (edge i -> partition i%128, chunk i//128). Tables > 32767 rows are
   split at HALF for the int16 index limit (two gathers per tile).
 - Segment-sum on the PE: DVE builds the per-tile indicator
   W[p, k, r] = val[p,k] * (row[p,k] == r) with two batched tensor_tensor
   ops (0-stride broadcast APs), and the PE accumulates W_k.T @ G_k into a
   PSUM tile over all chunks k (bf16 operands, fp32 accumulation).
 - msg/x tables are republished between phases with an 8-core AllGather.
 - Dropout masks are bit-exact jax-CPU threefry, computed host-side (they
   depend only on the fixed key/shape) and passed as inputs.
 - Output mean accumulates into the (zero-initialized) output via SWDGE
   accumulate-DMA of 0.25-scaled tiles.
"""

import os
import subprocess
import sys
import tempfile

import numpy as np

P = 128
D = 256
CORES = 8
NUM_LAYERS = 3
KEEP = 0.9

# real problem dims
N_POIS = 50000
N_HYPER = 50000
NNZ = 1600000
HALF_DEFAULT = 32768


# --------------------------------------------------------------------------
# host-side preprocessing
# --------------------------------------------------------------------------

def _snake_pack(dest, n_rows, n_bins):
    """Deal rows of one destination space into n_bins bins of 128 rows,
    balancing edge counts. Returns perm[row] -> global padded position
    (bin*128 + round)."""
    deg = np.bincount(dest, minlength=n_rows)
    order = np.argsort(-deg, kind="stable")
    padded = np.full(n_bins * P, -1, np.int64)
    padded[:n_rows] = order
    rounds = padded.reshape(P, n_bins).copy()
    rounds[1::2] = rounds[1::2, ::-1]          # snake
    perm = np.empty(n_rows, np.int64)
    rr, bb = np.meshgrid(np.arange(P), np.arange(n_bins), indexing="ij")
    valid = rounds >= 0
    perm[rounds[valid]] = (bb * P + rr)[valid]
    return perm


def _layout_edges(dest_g, col_g, val, n_bins, half):
    """Assign edges to (bin, chunk, partition) slots.

    dest_g: per-edge global padded destination position.
    col_g: per-edge global padded gather index (into the permuted table).
    Returns (idxlo_w, idxhi_w, row_s, val_s, C_LO, C_HI):
      idxlo_w [128, n_bins, 8*C_LO] int16   (16-wrapped, 8x replicated)
      idxhi_w [128, n_bins, 8*C_HI] int16
      row_s   [128, n_bins, C] f32          (C = C_LO + C_HI)
      val_s   [128, n_bins, C] f32
    """
    bin_e = dest_g // P
    part_r = (dest_g % P).astype(np.float64)
    is_hi = col_g >= half
    key = bin_e * 2 + is_hi
    order = np.argsort(key, kind="stable")
    key_s = key[order]
    counts = np.bincount(key_s, minlength=n_bins * 2)
    starts = np.concatenate([[0], np.cumsum(counts)[:-1]])
    rank = np.arange(len(order)) - starts[key_s]       # rank within segment

    lo_cnt = counts[0::2]
    hi_cnt = counts[1::2]
    c_lo = max(1, int(np.ceil(lo_cnt.max() / P)))
    c_hi = max(1, int(np.ceil(hi_cnt.max() / P)))
    c_tot = c_lo + c_hi

    idxlo_flat = np.zeros((n_bins, c_lo * P), np.int16)
    idxhi_flat = np.zeros((n_bins, c_hi * P), np.int16)
    row_cv = np.zeros((n_bins, c_tot, P), np.float32)
    val_cv = np.zeros((n_bins, c_tot, P), np.float32)

    e = order
    ki = key_s
    lo_m = (ki % 2) == 0
    b_lo = ki[lo_m] // 2
    r_lo = rank[lo_m]
    idxlo_flat[b_lo, r_lo] = col_g[e[lo_m]].astype(np.int16)
    row_cv[b_lo, r_lo // P, r_lo % P] = part_r[e[lo_m]]
    val_cv[b_lo, r_lo // P, r_lo % P] = val[e[lo_m]]

    hi_m = ~lo_m
    b_hi = ki[hi_m] // 2
    r_hi = rank[hi_m]
    idxhi_flat[b_hi, r_hi] = (col_g[e[hi_m]] - half).astype(np.int16)
    row_cv[b_hi, c_lo + r_hi // P, r_hi % P] = part_r[e[hi_m]]
    val_cv[b_hi, c_lo + r_hi // P, r_hi % P] = val[e[hi_m]]

    def wrap(flat, c):
        # [n_bins, c*128] -> [128, n_bins, 8c]: element i of bin b at
        # [i%16, b, i//16], replicated over the 8 16-partition groups
        a = flat.reshape(n_bins, c * 8, 16).transpose(2, 0, 1)
        return np.tile(a, (8, 1, 1))

    idxlo_w = wrap(idxlo_flat, c_lo)
    idxhi_w = wrap(idxhi_flat, c_hi)
    row_s = row_cv.transpose(2, 0, 1).astype(np.float32)
    val_s = val_cv.transpose(2, 0, 1).astype(np.float32)
    return idxlo_w, idxhi_w, row_s, val_s, c_lo, c_hi


_MASK_CODE = """
import numpy as np, jax
import sys
n, d, layers = int(sys.argv[1]), int(sys.argv[2]), int(sys.argv[3])
key = jax.random.key(42)
out = np.empty((layers, n, d), np.bool_)
for i in range(layers):
    out[i] = np.asarray(jax.random.bernoulli(jax.random.fold_in(key, i), 0.9, (n, d)))
np.save(sys.argv[4], np.packbits(out, axis=-1))
"""


def _compute_masks(n, d, layers):
    """Dropout masks, bit-exact with the reference (jax threefry on CPU)."""
    with tempfile.TemporaryDirectory() as td:
        path = os.path.join(td, "m.npy")
        env = dict(os.environ)
        env["JAX_PLATFORMS"] = "cpu"
        subprocess.run(
            [sys.executable, "-c", _MASK_CODE, str(n), str(d), str(layers), path],
            check=True, env=env, capture_output=True,
        )
        packed = np.load(path)
    return np.unpackbits(packed, axis=-1).astype(bool)[:, :, :d]


# --------------------------------------------------------------------------
# device kernel builder
# --------------------------------------------------------------------------

def _build_nc(T, TA, ct_lo, ct_hi, cs_lo, cs_hi, half, gtot, use_bf16):
    import concourse.bacc as bacc
    import concourse.mybir as mybir
    import concourse.tile as tile

    f32 = mybir.dt.float32
    gdt = mybir.dt.bfloat16 if use_bf16 else f32
    i16 = mybir.dt.int16
    RPC = T * P
    ct = ct_lo + ct_hi
    cs = cs_lo + cs_hi

    nc = bacc.Bacc("TRN2", target_bir_lowering=False, debug=False,
                   num_devices=CORES, num_swdge_queues=4)

    def din(name, shape, dt):
        return nc.dram_tensor(name, shape, dt, kind="ExternalInput")

    x0_full = din("x0_full", [gtot, D], gdt)
    x0_slice = din("x0_slice", [RPC, D], f32)
    masks_d = din("masks", [NUM_LAYERS, RPC, D], f32)
    cmax = max(ct, cs)
    iota_d = din("iota", [P, cmax * P], gdt)
    it_lo = din("it_lo", [P, T * 8 * ct_lo], i16)
    it_hi = din("it_hi", [P, T * 8 * ct_hi], i16)
    rt_d = din("rt", [P, T * ct], gdt)
    vt_d = din("vt", [P, T * ct], gdt)
    is_lo = din("is_lo", [P, T * 8 * cs_lo], i16)
    is_hi = din("is_hi", [P, T * 8 * cs_hi], i16)
    rs_d = din("rs", [P, T * cs], gdt)
    vs_d = din("vs", [P, T * cs], gdt)

    xouts = [nc.dram_tensor(f"xo{l}", [RPC, D], f32, kind="ExternalOutput")
             for l in range(NUM_LAYERS)]

    msg_loc = nc.dram_tensor("msg_loc", [RPC, D], gdt)
    msg_full = nc.dram_tensor("msg_full", [gtot, D], gdt, addr_space="Shared")
    x_full = nc.dram_tensor("x_full", [gtot, D], gdt, addr_space="Shared")
    if use_bf16:
        xl_locb = nc.dram_tensor("xl_locb", [RPC, D], gdt)
    else:
        xl_locb = xouts  # per-layer f32 buffers double as AG inputs

    rg = [list(range(CORES))]

    with tile.TileContext(nc) as tc, \
         tc.tile_pool(name="res", bufs=1) as res, \
         tc.tile_pool(name="gp", bufs=3) as gp, \
         tc.tile_pool(name="wp", bufs=2) as wp, \
         tc.tile_pool(name="sm", bufs=3) as sm, \
         tc.tile_pool(name="pp", bufs=4, space="PSUM") as pp:

        # resident SBUF data
        iota_sb = res.tile([P, cmax * P], gdt)
        nc.sync.dma_start(iota_sb[:], iota_d[:, :])
        ed = {}
        for tag, (ilo, ihi, rr, vv, clo, chi, c) in {
            "t": (it_lo, it_hi, rt_d, vt_d, ct_lo, ct_hi, ct),
            "s": (is_lo, is_hi, rs_d, vs_d, cs_lo, cs_hi, cs),
        }.items():
            ilo_sb = res.tile([P, T * 8 * clo], i16, tag=f"ilo{tag}")
            ihi_sb = res.tile([P, T * 8 * chi], i16, tag=f"ihi{tag}")
            r_sb = res.tile([P, T * c], gdt, tag=f"r{tag}")
            v_sb = res.tile([P, T * c], gdt, tag=f"v{tag}")
            nc.sync.dma_start(ilo_sb[:], ilo[:, :])
            nc.sync.dma_start(ihi_sb[:], ihi[:, :])
            nc.sync.dma_start(r_sb[:], rr[:, :])
            nc.sync.dma_start(v_sb[:], vv[:, :])
            ed[tag] = (ilo_sb, ihi_sb, r_sb, v_sb, clo, chi, c)

        MAXC = 8     # max chunks (1024 indices) per dma_gather instruction
        qn = [0]     # SWDGE queue rotation

        def spmm_tile(tag, table, t, post):
            ilo_sb, ihi_sb, r_sb, v_sb, clo, chi, c = ed[tag]
            g = gp.tile([P, c, D], gdt, tag="g")
            for base, cnt, tab, idx_sb in (
                (0, clo, table[:half, :], ilo_sb),
                (clo, chi, table[half:, :], ihi_sb),
            ):
                cn = cnt
                for c0 in range(0, cn, MAXC):
                    cw = min(MAXC, cn - c0)
                    nc.gpsimd.dma_gather(
                        out_ap=g[:, base + c0:base + c0 + cw, :],
                        in_ap=tab,
                        idxs_ap=idx_sb[:, t * 8 * cn + 8 * c0:
                                       t * 8 * cn + 8 * (c0 + cw)],
                        num_idxs=cw * P,
                        num_idxs_reg=cw * P,
                        elem_size=D,
                        queue_num=qn[0] % 4,
                    )
                    qn[0] += 1
            # batched indicator build: W[p, k, r] = val[p,k]*(iota[r]==row[p,k])
            w = wp.tile([P, c, P], gdt, tag="w")
            nc.vector.tensor_tensor(
                out=w[:, :, :],
                in0=iota_sb[:, :c * P].rearrange("p (c r) -> p c r", r=P),
                in1=r_sb[:, t * c:(t + 1) * c].to_broadcast((P, c, P)),
                op=mybir.AluOpType.is_equal,
            )
            nc.vector.tensor_tensor(
                out=w[:, :, :],
                in0=w[:, :, :],
                in1=v_sb[:, t * c:(t + 1) * c].to_broadcast((P, c, P)),
                op=mybir.AluOpType.mult,
            )
            ps = pp.tile([P, D], f32, space="PSUM", tag="ps")
            for k in range(c):
                nc.tensor.matmul(
                    out=ps[:],
                    lhsT=w[:, k, :],
                    rhs=g[:, k, :],
                    start=(k == 0),
                    stop=(k == c - 1),
                )
            post(t, ps)

        for l in range(NUM_LAYERS):
            xtab = x0_full if l == 0 else x_full

            def post_tar(t, ps):
                msb = sm.tile([P, D], gdt, tag="msb")
                nc.scalar.copy(msb[:], ps[:])
                nc.sync.dma_start(msg_loc[t * P:(t + 1) * P, :], msb[:])

            for t in range(T):
                spmm_tile("t", xtab, t, post_tar)
                if t == TA - 1:
                    nc.gpsimd.collective_compute(
                        "AllGather", mybir.AluOpType.bypass,
                        replica_groups=rg,
                        ins=[msg_loc[0:TA * P, :].opt()],
                        outs=[msg_full[0:CORES * TA * P, :].opt()],
                    )
            nc.gpsimd.collective_compute(
                "AllGather", mybir.AluOpType.bypass, replica_groups=rg,
                ins=[msg_loc[TA * P:T * P, :].opt()],
                outs=[msg_full[CORES * TA * P:CORES * T * P, :].opt()],
            )

            def post_src(t, ps, l=l):
                xprev = sm.tile([P, D], f32, tag="xprev")
                src_prev = x0_slice if l == 0 else xouts[l - 1]
                nc.sync.dma_start(xprev[:], src_prev[t * P:(t + 1) * P, :])
                mk = sm.tile([P, D], f32, tag="mk")
                nc.sync.dma_start(mk[:], masks_d[l, t * P:(t + 1) * P, :])
                xn = sm.tile([P, D], f32, tag="xn")
                nc.vector.tensor_tensor(out=xn[:], in0=ps[:], in1=xprev[:],
                                        op=mybir.AluOpType.add)
                nc.vector.tensor_tensor(out=xn[:], in0=xn[:], in1=mk[:],
                                        op=mybir.AluOpType.mult)
                nc.sync.dma_start(xouts[l][t * P:(t + 1) * P, :], xn[:])
                if l < NUM_LAYERS - 1 and use_bf16:
                    xnb = sm.tile([P, D], gdt, tag="xnb")
                    nc.scalar.copy(xnb[:], xn[:])
                    nc.sync.dma_start(xl_locb[t * P:(t + 1) * P, :],
                                      xnb[:])

            for t in range(T):
                spmm_tile("s", msg_full, t, post_src)
                if t == TA - 1 and l < NUM_LAYERS - 1:
                    agsrc = xl_locb if use_bf16 else xouts[l]
                    nc.gpsimd.collective_compute(
                        "AllGather", mybir.AluOpType.bypass,
                        replica_groups=rg,
                        ins=[agsrc[0:TA * P, :].opt()],
                        outs=[x_full[0:CORES * TA * P, :].opt()],
                    )
            if l < NUM_LAYERS - 1:
                agsrc = xl_locb if use_bf16 else xouts[l]
                nc.gpsimd.collective_compute(
                    "AllGather", mybir.AluOpType.bypass, replica_groups=rg,
                    ins=[agsrc[TA * P:T * P, :].opt()],
                    outs=[x_full[CORES * TA * P:CORES * T * P, :].opt()],
                )

    nc.compile()
    return nc


# --------------------------------------------------------------------------
# public entry point
# --------------------------------------------------------------------------

def _run(poi_embs, src_row, src_col, src_val, tar_row, tar_col, tar_val,
         n_pois, n_hyper, use_bf16=True, trace=False):
    from concourse.bass_utils import run_bass_kernel_spmd

    n_bins_n = -(-n_pois // P)
    n_bins_n = -(-n_bins_n // CORES) * CORES     # multiple of CORES
    n_bins_h = -(-n_hyper // P)
    n_bins_h = -(-n_bins_h // CORES) * CORES
    n_bins = max(n_bins_n, n_bins_h)             # same T for both phases
    T = n_bins // CORES
    RPC = T * P
    gtot = n_bins * P
    half = min(HALF_DEFAULT, (gtot // 2 + 255) & ~255)

    TA = (T + 1) // 2
    TB = T - TA
    # global layout supporting split AllGathers: half A = tiles [0,TA) of
    # every core (rank-major), then half B = tiles [TA,T)
    cc, tt = np.meshgrid(np.arange(CORES), np.arange(T), indexing="ij")
    bases = np.where(tt < TA, (cc * TA + tt) * P,
                     CORES * TA * P + (cc * TB + (tt - TA)) * P).reshape(-1)
    gmap = (bases[:, None] + np.arange(P)[None, :]).reshape(-1)
    l2g = gmap.reshape(CORES, T * P)          # per-core local row -> global
    perm_n = gmap[_snake_pack(src_row, n_pois, n_bins)]    # POI space
    perm_h = gmap[_snake_pack(tar_row, n_hyper, n_bins)]   # hyperedge space

    inv_gmap = np.empty_like(gmap)
    inv_gmap[gmap] = np.arange(gtot)
    packbin_n = inv_gmap[perm_n]     # bin-major position (core*T+t)*P + r
    packbin_h = inv_gmap[perm_h]
    # tar-SpMM: dest in H-space, gathers from POI table
    it_lo, it_hi, rt, vt, ct_lo, ct_hi = _layout_edges(
        packbin_h[tar_row], perm_n[tar_col], tar_val, n_bins, half)
    # src-SpMM: dest in N-space, gathers from hyperedge (msg) table
    is_lo, is_hi, rs, vs, cs_lo, cs_hi = _layout_edges(
        packbin_n[src_row], perm_h[src_col], src_val, n_bins, half)

    x0_full = np.zeros((gtot, D), np.float32)
    x0_full[perm_n] = poi_embs
    masks = _compute_masks(n_pois, D, NUM_LAYERS)
    mask_scaled = np.zeros((NUM_LAYERS, gtot, D), np.float32)
    mask_scaled[:, perm_n] = masks.astype(np.float32) * np.float32(1.0 / KEEP)

    cmax = max(ct_lo + ct_hi, cs_lo + cs_hi)
    iota = np.broadcast_to(
        np.tile(np.arange(P, dtype=np.float32), cmax), (P, cmax * P)).copy()

    if use_bf16:
        import ml_dtypes
        bdt = ml_dtypes.bfloat16
        x0_tab = x0_full.astype(bdt)
        iota = iota.astype(bdt)
        rt, vt, rs, vs = (a.astype(bdt) for a in (rt, vt, rs, vs))
    else:
        x0_tab = x0_full

    in_maps = []
    for c in range(CORES):
        bs = slice(c * T, (c + 1) * T)
        rows = l2g[c]
        in_maps.append({
            "x0_full": x0_tab,
            "x0_slice": np.ascontiguousarray(x0_full[rows]),
            "masks": np.ascontiguousarray(mask_scaled[:, rows]),
            "iota": iota,
            "it_lo": np.ascontiguousarray(it_lo[:, bs]).reshape(P, -1),
            "it_hi": np.ascontiguousarray(it_hi[:, bs]).reshape(P, -1),
            "rt": np.ascontiguousarray(rt[:, bs]).reshape(P, -1),
            "vt": np.ascontiguousarray(vt[:, bs]).reshape(P, -1),
            "is_lo": np.ascontiguousarray(is_lo[:, bs]).reshape(P, -1),
            "is_hi": np.ascontiguousarray(is_hi[:, bs]).reshape(P, -1),
            "rs": np.ascontiguousarray(rs[:, bs]).reshape(P, -1),
            "vs": np.ascontiguousarray(vs[:, bs]).reshape(P, -1),
        })

    nc = _build_nc(T, TA, ct_lo, ct_hi, cs_lo, cs_hi, half, gtot, use_bf16)
    kw = {"trace": True} if trace else {}
    res = run_bass_kernel_spmd(nc, in_maps, core_ids=list(range(CORES)), **kw)

    full = np.empty((gtot, D), np.float32)
    for c in range(CORES):
        r = res.results[c]
        full[l2g[c]] = 0.25 * (x0_full[l2g[c]] + r["xo0"] + r["xo1"]
                               + r["xo2"])
    out = full[perm_n]
    return out.astype(np.float32), res


def kernel(poi_embs, src_row, src_col, src_val, tar_row, tar_col, tar_val,
           num_pois, num_hyperedges, **_ignored):
    out, _ = _run(
        np.asarray(poi_embs, np.float32),
        np.asarray(src_row).astype(np.int64),
        np.asarray(src_col).astype(np.int64),
        np.asarray(src_val, np.float32),
        np.asarray(tar_row).astype(np.int64),
        np.asarray(tar_col).astype(np.int64),
        np.asarray(tar_val, np.float32),
        int(num_pois), int(num_hyperedges),
        use_bf16=True,
    )
    return out
